# revision 5
# baseline (speedup 1.0000x reference)
"""LSEP loss kernel for Trainium2, data-parallel over 8 NeuronCores.

Math per element i (B=1e6, C=10):
  q[c]  = T[i, bayes[i], c]
  s_neg = sum_c (partial[i,c]==0) * exp(q[c])
  s_pos = sum_c (partial[i,c]==1) * exp(-q[c])
  loss  = mean_i log1p(s_neg * s_pos)

Strategy: shard i across the 8 cores. Per core, tiles of 128 partitions x
N_PER_PART elements; each element's 10x10 T block is 100 contiguous f32 in
one partition. Row selection is done without any gather: gpsimd multiplies
the T tile in place by onehot(bayes[i]) broadcast over c (a stride-0 AP
view of the [j,r] mask), then a DVE reduce-add over r yields q exactly
(one nonzero per (j,c)). The per-core scalar sum of log1p terms comes back
and the host sums 8 scalars and divides by B.
"""

from contextlib import ExitStack

import numpy as np

import concourse.bacc as bacc
import concourse.bass_isa as bass_isa
import concourse.mybir as mybir
import concourse.tile as tile
from concourse.bass_utils import run_bass_kernel_spmd

f32 = mybir.dt.float32
i32 = mybir.dt.int32
Alu = mybir.AluOpType
Act = mybir.ActivationFunctionType
Axis = mybir.AxisListType

BIG = 1024.0
C = 10
CC = C * C

B = 1_000_000
NCORES = 8
N_PER_PART = 70
N_TILES = 14
B_CORE = 128 * N_PER_PART * N_TILES  # 125440
assert B_CORE * NCORES >= B


def build_core_program(nc, n: int, ntiles: int):
    """Emit the per-core program into `nc` (a Bacc). Dram tensors:
    t_in [b,100] f32, bayes_in [b] f32, partial_in [b,10] f32,
    sum_out [1,1] f32, where b = 128*n*ntiles."""
    b = 128 * n * ntiles
    T_d = nc.dram_tensor("t_in", [b, CC], f32, kind="ExternalInput").ap()
    bay_d = nc.dram_tensor("bayes_in", [b], f32, kind="ExternalInput").ap()
    par_d = nc.dram_tensor("partial_in", [b, C], f32, kind="ExternalInput").ap()
    out_d = nc.dram_tensor("sum_out", [1, 1], f32, kind="ExternalOutput").ap()

    T_v = T_d.rearrange("(t p n) c -> t p (n c)", t=ntiles, p=128, n=n)
    bay_v = bay_d.rearrange("(t p n) -> t p n", t=ntiles, p=128, n=n)
    par_v = par_d.rearrange("(t p n) c -> t p (n c)", t=ntiles, p=128, n=n)

    with tile.TileContext(nc) as tc, ExitStack() as ctx:
        const_pool = ctx.enter_context(tc.tile_pool(name="const", bufs=1))
        big_pool = ctx.enter_context(tc.tile_pool(name="big", bufs=3))
        small_pool = ctx.enter_context(tc.tile_pool(name="small", bufs=3))
        acc_pool = ctx.enter_context(tc.tile_pool(name="acc", bufs=1))

        iota_i = const_pool.tile([128, C], i32)
        nc.gpsimd.iota(iota_i[:], pattern=[[1, C]], base=0, channel_multiplier=0)
        iota_f = const_pool.tile([128, C], f32)
        nc.vector.tensor_copy(iota_f[:], iota_i[:])

        prodbuf = acc_pool.tile([128, ntiles * n], f32)

        for t in range(ntiles):
            tB = small_pool.tile([128, n], f32, tag="bayes")
            nc.sync.dma_start(tB[:], bay_v[t])

            # mask10[p, j, r] = (bayes[p,j] == r)
            tM = small_pool.tile([128, C * n], f32, tag="mask")
            nc.vector.tensor_tensor(
                tM[:].rearrange("p (j r) -> p j r", j=n),
                tB[:].unsqueeze(2).broadcast_to([128, n, C]),
                iota_f[:].unsqueeze(1).broadcast_to([128, n, C]),
                op=Alu.is_equal,
            )

            # plain fast T load
            tT = big_pool.tile([128, CC * n], f32, tag="tbuf")
            nc.sync.dma_start(tT[:], T_v[t])

            # row selection in place: T *= onehot(bayes) broadcast over c
            nc.gpsimd.tensor_tensor(
                tT[:].rearrange("p (j r c) -> p j r c", j=n, r=C),
                tM[:].rearrange("p (j r) -> p j r", j=n)
                .unsqueeze(3)
                .broadcast_to([128, n, C, C]),
                tT[:].rearrange("p (j r c) -> p j r c", j=n, r=C),
                op=Alu.mult,
            )

            # q[p, j, c] = sum_r qsel (exact: one nonzero per (j,c))
            tQ = small_pool.tile([128, C * n], f32, tag="q")
            nc.vector.tensor_reduce(
                tQ[:].rearrange("p (j c) -> p j c", j=n),
                tT[:].rearrange("p (j r c) -> p j c r", j=n, r=C),
                axis=Axis.X,
                op=Alu.add,
            )

            tEq = small_pool.tile([128, C * n], f32, tag="eq")
            nc.scalar.activation(tEq[:], tQ[:], Act.Exp, scale=1.0)
            tEn = small_pool.tile([128, C * n], f32, tag="en")
            nc.scalar.activation(tEn[:], tQ[:], Act.Exp, scale=-1.0)

            # partial masks: pos = partial itself, neg = (partial == 0)
            tP = small_pool.tile([128, C * n], f32, tag="part")
            nc.sync.dma_start(tP[:], par_v[t])
            tNeg = small_pool.tile([128, C * n], f32, tag="neg")
            nc.gpsimd.tensor_scalar(tNeg[:], tP[:], 0.0, None, op0=Alu.is_equal)

            # s_neg = sum_c neg*eq ; s_pos = sum_c pos*enq
            nc.vector.tensor_tensor(tEq[:], tEq[:], tNeg[:], op=Alu.mult)
            tS0 = small_pool.tile([128, n], f32, tag="sneg")
            nc.vector.tensor_reduce(
                tS0[:], tEq[:].rearrange("p (j c) -> p j c", j=n), axis=Axis.X, op=Alu.add
            )
            nc.vector.tensor_tensor(tEn[:], tEn[:], tP[:], op=Alu.mult)
            tS1 = small_pool.tile([128, n], f32, tag="spos")
            nc.vector.tensor_reduce(
                tS1[:], tEn[:].rearrange("p (j c) -> p j c", j=n), axis=Axis.X, op=Alu.add
            )

            nc.vector.tensor_tensor(
                prodbuf[:, t * n : (t + 1) * n], tS0[:], tS1[:], op=Alu.mult
            )

        # epilogue: log1p, row-sum, partition-sum, dma out
        termbuf = acc_pool.tile([128, ntiles * n], f32)
        nc.scalar.activation(termbuf[:], prodbuf[:], Act.Ln, bias=1.0, scale=1.0)
        colsum = acc_pool.tile([128, 1], f32)
        nc.vector.tensor_reduce(colsum[:], termbuf[:], axis=Axis.X, op=Alu.add)
        total = acc_pool.tile([128, 1], f32)
        nc.gpsimd.partition_all_reduce(
            total[:], colsum[:], channels=128, reduce_op=bass_isa.ReduceOp.add
        )
        nc.sync.dma_start(out_d, total[:1, :])

    nc.compile()
    return nc


_PROGRAM_CACHE = {}


def _get_program():
    key = (N_PER_PART, N_TILES)
    if key not in _PROGRAM_CACHE:
        nc = bacc.Bacc("TRN2", target_bir_lowering=False, debug=False)
        build_core_program(nc, N_PER_PART, N_TILES)
        _PROGRAM_CACHE[key] = nc
    return _PROGRAM_CACHE[key]


def kernel(T, bayes, partial, _trace=False):
    assert T.shape == (B, C, C) and bayes.shape == (B,) and partial.shape == (B, C)
    Tf = np.ascontiguousarray(T, dtype=np.float32).reshape(B, CC)
    bayf = np.asarray(bayes).astype(np.float32)
    parf = np.asarray(partial).astype(np.float32)

    in_maps = []
    for k in range(NCORES):
        lo, hi = k * B_CORE, min((k + 1) * B_CORE, B)
        tk = Tf[lo:hi]
        bk = bayf[lo:hi]
        pk = parf[lo:hi]
        pad = B_CORE - (hi - lo)
        if pad > 0:
            # padded elements contribute exactly 0: partial=1 everywhere
            # makes s_neg = 0 so log1p(0) = 0
            tk = np.concatenate([tk, np.zeros((pad, CC), np.float32)])
            bk = np.concatenate([bk, np.zeros((pad,), np.float32)])
            pk = np.concatenate([pk, np.ones((pad, C), np.float32)])
        in_maps.append({"t_in": tk, "bayes_in": bk, "partial_in": pk})

    nc = _get_program()
    res = run_bass_kernel_spmd(
        nc, in_maps, core_ids=list(range(NCORES)), trace=_trace
    )
    total = sum(float(res.results[k]["sum_out"][0, 0]) for k in range(NCORES))
    out = np.float32(total / B)
    if _trace:
        return out, res
    return out


# revision 6
# speedup vs baseline: 1.4164x; 1.4164x over previous
"""LSEP loss kernel for Trainium2, data-parallel over 8 NeuronCores.

Math per element i (B=1e6, C=10):
  q[c]  = T[i, bayes[i], c]
  s_neg = sum_c (partial[i,c]==0) * exp(q[c])
  s_pos = sum_c (partial[i,c]==1) * exp(-q[c])
  loss  = mean_i log1p(s_neg * s_pos)

Strategy: shard i across the 8 cores. Per core, tiles of 128 partitions x
N_PER_PART elements; each element's 10x10 T block is 100 contiguous f32 in
one partition, staged host-side as T^T blocks (r innermost) so every DVE
pass is packed-unit-stride. Row selection without any gather: gpsimd
multiplies the T tile in place by onehot(bayes[i]) broadcast over c (a
stride-0 AP view of the [j,r] mask), then a DVE reduce-add over the
innermost r yields q exactly (one nonzero per (j,c)). Per-core [128,1]
partial sums of the log1p terms come back; the host sums and divides by B.
"""

from contextlib import ExitStack

import numpy as np

import concourse.bacc as bacc
import concourse.bass_isa as bass_isa
import concourse.mybir as mybir
import concourse.tile as tile
from concourse.bass_utils import run_bass_kernel_spmd

f32 = mybir.dt.float32
i32 = mybir.dt.int32
Alu = mybir.AluOpType
Act = mybir.ActivationFunctionType
Axis = mybir.AxisListType

BIG = 1024.0
C = 10
CC = C * C

B = 1_000_000
NCORES = 8
N_PER_PART = 70
N_TILES = 14
B_CORE = 128 * N_PER_PART * N_TILES  # 125440
assert B_CORE * NCORES >= B


def build_core_program(nc, n: int, ntiles: int):
    """Emit the per-core program into `nc` (a Bacc). Dram tensors:
    t_in [b,100] f32, bayes_in [b] f32, partial_in [b,10] f32,
    sum_out [1,1] f32, where b = 128*n*ntiles."""
    b = 128 * n * ntiles
    T_d = nc.dram_tensor("t_in", [b, CC], f32, kind="ExternalInput").ap()
    bay_d = nc.dram_tensor("bayes_in", [b], f32, kind="ExternalInput").ap()
    par_d = nc.dram_tensor("partial_in", [b, C], f32, kind="ExternalInput").ap()
    out_d = nc.dram_tensor("sum_out", [128, 1], f32, kind="ExternalOutput").ap()

    T_v = T_d.rearrange("(t p n) c -> t p (n c)", t=ntiles, p=128, n=n)
    bay_v = bay_d.rearrange("(t p n) -> t p n", t=ntiles, p=128, n=n)
    par_v = par_d.rearrange("(t p n) c -> t p (n c)", t=ntiles, p=128, n=n)

    with tile.TileContext(nc) as tc, ExitStack() as ctx:
        const_pool = ctx.enter_context(tc.tile_pool(name="const", bufs=1))
        big_pool = ctx.enter_context(tc.tile_pool(name="big", bufs=3))
        small_pool = ctx.enter_context(tc.tile_pool(name="small", bufs=3))
        acc_pool = ctx.enter_context(tc.tile_pool(name="acc", bufs=1))

        iota_i = const_pool.tile([128, C], i32)
        nc.gpsimd.iota(iota_i[:], pattern=[[1, C]], base=0, channel_multiplier=0)
        iota_f = const_pool.tile([128, C], f32)
        nc.vector.tensor_copy(iota_f[:], iota_i[:])

        prodbuf = acc_pool.tile([128, ntiles * n], f32)

        for t in range(ntiles):
            tB = small_pool.tile([128, n], f32, tag="bayes")
            nc.sync.dma_start(tB[:], bay_v[t])

            # mask10[p, j, r] = (bayes[p,j] == r)
            tM = small_pool.tile([128, C * n], f32, tag="mask")
            nc.vector.tensor_tensor(
                tM[:].rearrange("p (j r) -> p j r", j=n),
                tB[:].unsqueeze(2).broadcast_to([128, n, C]),
                iota_f[:].unsqueeze(1).broadcast_to([128, n, C]),
                op=Alu.is_equal,
            )

            # plain fast T load
            tT = big_pool.tile([128, CC * n], f32, tag="tbuf")
            nc.sync.dma_start(tT[:], T_v[t])

            # row selection in place (T staged as [j, c, r], r innermost):
            # T *= onehot(bayes) with the [j,r] mask broadcast over middle c
            nc.gpsimd.tensor_tensor(
                tT[:].rearrange("p (j c r) -> p j c r", j=n, c=C),
                tM[:].rearrange("p (j r) -> p j r", j=n)
                .unsqueeze(2)
                .broadcast_to([128, n, C, C]),
                tT[:].rearrange("p (j c r) -> p j c r", j=n, c=C),
                op=Alu.mult,
            )

            # q[p, j, c] = sum_r qsel (exact: one nonzero per (j,c)),
            # packed unit-stride innermost reduce
            tQ = small_pool.tile([128, C * n], f32, tag="q")
            nc.vector.tensor_reduce(
                tQ[:].rearrange("p (j c) -> p j c", j=n),
                tT[:].rearrange("p (j c r) -> p j c r", j=n, c=C),
                axis=Axis.X,
                op=Alu.add,
            )

            tEq = small_pool.tile([128, C * n], f32, tag="eq")
            nc.scalar.activation(tEq[:], tQ[:], Act.Exp, scale=1.0)
            tEn = small_pool.tile([128, C * n], f32, tag="en")
            nc.scalar.activation(tEn[:], tQ[:], Act.Exp, scale=-1.0)

            # partial masks: pos = partial itself, neg = (partial == 0)
            tP = small_pool.tile([128, C * n], f32, tag="part")
            nc.sync.dma_start(tP[:], par_v[t])
            tNeg = small_pool.tile([128, C * n], f32, tag="neg")
            nc.vector.tensor_scalar(tNeg[:], tP[:], 0.0, None, op0=Alu.is_equal)

            # s_neg = sum_c neg*eq ; s_pos = sum_c pos*enq
            nc.vector.tensor_tensor(tEq[:], tEq[:], tNeg[:], op=Alu.mult)
            tS0 = small_pool.tile([128, n], f32, tag="sneg")
            nc.vector.tensor_reduce(
                tS0[:], tEq[:].rearrange("p (j c) -> p j c", j=n), axis=Axis.X, op=Alu.add
            )
            nc.vector.tensor_tensor(tEn[:], tEn[:], tP[:], op=Alu.mult)
            tS1 = small_pool.tile([128, n], f32, tag="spos")
            nc.vector.tensor_reduce(
                tS1[:], tEn[:].rearrange("p (j c) -> p j c", j=n), axis=Axis.X, op=Alu.add
            )

            nc.vector.tensor_tensor(
                prodbuf[:, t * n : (t + 1) * n], tS0[:], tS1[:], op=Alu.mult
            )

        # epilogue: log1p, row-sum, partition-sum, dma out
        termbuf = acc_pool.tile([128, ntiles * n], f32)
        nc.scalar.activation(termbuf[:], prodbuf[:], Act.Ln, bias=1.0, scale=1.0)
        colsum = acc_pool.tile([128, 1], f32)
        nc.vector.tensor_reduce(colsum[:], termbuf[:], axis=Axis.X, op=Alu.add)
        nc.sync.dma_start(out_d, colsum[:])

    nc.compile()
    return nc


_PROGRAM_CACHE = {}


def _get_program():
    key = (N_PER_PART, N_TILES)
    if key not in _PROGRAM_CACHE:
        nc = bacc.Bacc("TRN2", target_bir_lowering=False, debug=False)
        build_core_program(nc, N_PER_PART, N_TILES)
        _PROGRAM_CACHE[key] = nc
    return _PROGRAM_CACHE[key]


def kernel(T, bayes, partial, _trace=False):
    assert T.shape == (B, C, C) and bayes.shape == (B,) and partial.shape == (B, C)
    # stage T as transposed blocks [i, c, r] so the on-device r-reduce is
    # unit-stride innermost
    Tf = np.ascontiguousarray(
        np.asarray(T, dtype=np.float32).reshape(B, C, C).transpose(0, 2, 1)
    ).reshape(B, CC)
    bayf = np.asarray(bayes).astype(np.float32)
    parf = np.asarray(partial).astype(np.float32)

    in_maps = []
    for k in range(NCORES):
        lo, hi = k * B_CORE, min((k + 1) * B_CORE, B)
        tk = Tf[lo:hi]
        bk = bayf[lo:hi]
        pk = parf[lo:hi]
        pad = B_CORE - (hi - lo)
        if pad > 0:
            # padded elements contribute exactly 0: partial=1 everywhere
            # makes s_neg = 0 so log1p(0) = 0
            tk = np.concatenate([tk, np.zeros((pad, CC), np.float32)])
            bk = np.concatenate([bk, np.zeros((pad,), np.float32)])
            pk = np.concatenate([pk, np.ones((pad, C), np.float32)])
        in_maps.append({"t_in": tk, "bayes_in": bk, "partial_in": pk})

    nc = _get_program()
    res = run_bass_kernel_spmd(
        nc, in_maps, core_ids=list(range(NCORES)), trace=_trace
    )
    total = sum(
        float(res.results[k]["sum_out"].astype(np.float64).sum())
        for k in range(NCORES)
    )
    out = np.float32(total / B)
    if _trace:
        return out, res
    return out
